# revision 28
# baseline (speedup 1.0000x reference)
"""Trainium2 Bass kernel for nn_CA_event (CA_event.forward batched ODE RHS).

reference:
    x   = state[:, 0:100]
    e_x = state[:, 100:200]
    W_a = state[:, 300:400]          (W_c = state[:, 200:300] unused)
    u   = W_a * (x + e_x - target)
    s   = x^2 / (1 + x^2)
    dx  = -x + s @ A.T + u * s
    out = concat([dx, -dx, 0, 0], axis=-1)      # [B, 400]

Strategy: pure data parallel over 8 NeuronCores (batch 131072 -> 16384
rows/core), FEATURE-MAJOR bf16 layout.  Host packs the three used state
slices transposed into one bf16 tensor xew[c] = [3, 100, 16384]
(planes x / e_x / W_a; feature dim on partitions), sends the constants
nAT = -A.T (bf16) and ntgt = -target ([100,1] f32).  The rel-err gate is
2e-2; bf16 I/O keeps the L2 error ~3e-3 while cutting HBM traffic from
2000 B/row (f32, both dx and -dx stored) to 800 B/row: reads 600 B
(x,e,W_a bf16) + writes 200 B (dx bf16 only; -dx is mirrored host-side,
the W_c/W_a derivative halves are structurally zero for any input).

Per 2048-row tile [100 partitions x 2048] (default knobs):
    V:   rm1 = 1/(1+x^2) - 1 = -s       (custom DVE op, 1x: NOT-seed + 1 NR)
         he2 = x + e                    (bf16 tensor_tensor, 2x mode)
         hm  = he2 + (-target)          (tensor_scalar, per-partition scalar)
         u   = hm * w                   (2x)
         t2  = u * rm1 = -u*s           (2x)
         v2  = t2 + x                   (2x)
    ACT: out = Copy(psum) -> bf16       (the only PSUM read)
    PE:  psum = nAT@rm1 + nI@v2 = A@s + u*s - x = dx^T
         (512-col chunks; no transposes anywhere -- the feature-major
          layout makes the moving operands k-major natively)
    DMA: x on the sync HWDGE queue (lands first; unblocks rm1 + nAT
         matmuls), e on the scalar HWDGE queue, W_a + stores on GpSimd
         SWDGE (the only queue that spreads over all 16 SDMA engines).
    For_i timing loop unrolls 3 passes per iteration to amortize the
    loop-boundary drain + semaphore reset (~8us).

Measured on the 8-core axon TRN2: ~52-57us/pass (vs 107-132us baseline);
DVE is the saturated engine (~97%), at its 5-op floor for this math.
"""

import os
import sys

try:
    import concourse  # noqa: F401  (resolves via the environment's default path)
except ImportError:  # fall back for bare environments
    sys.path.insert(0, "/opt/trn_rl_repo")

import numpy as np
import ml_dtypes

import concourse.bass as bass
import concourse.bacc as bacc
import concourse.mybir as mybir
from concourse import tile
from concourse import masks

DIM = 100
BATCH = 131072
NCORES = 8
ROWS_PER_CORE = BATCH // NCORES          # 16384

F32 = mybir.dt.float32
BF16 = mybir.dt.bfloat16
NP_BF16 = ml_dtypes.bfloat16

_RUNNERS = {}  # key -> runner dict
_CA_OPS = None


def _register_ca_ops():
    """Register the fused custom-DVE op rm1 = 1/(1+x^2) - 1 (= -s) from x.

    Chebyshev bitwise-NOT reciprocal seed + one Newton pass on d = 1+x^2,
    minus 1; ~1e-3 rel accuracy on r, which is plenty under the 2e-2 gate.
    Same body as the baseline's proven CA_RM1_NR1.
    """
    global _CA_OPS
    if _CA_OPS is not None:
        return _CA_OPS
    from concourse import dve_ops
    from concourse.dve_spec import Spec, Src0, C0, C1, One, Bin, AluOp, sq
    from concourse.dve_uop import DveOpSpec

    dC = sq(Src0) + One
    ndC = Bin(AluOp.BITWISE_NOT, dC, dC)
    y0C = ndC * C0
    bodyC = y0C * (C1 - dC * y0C) - One

    def refC(in0, in1, s0, s1, imm2):
        d = (1.0 + in0.astype(np.float32) * in0).astype(np.float32)
        nd = (~d.view(np.int32)).view(np.float32)
        yy0 = (nd * np.float32(s0)).astype(np.float32)
        return (yy0 * (np.float32(s1) - d * yy0) - 1.0).astype(np.float32)

    name, spec = "CA_RM1_NR1", Spec(body=bodyC, reference=refC)
    if name not in dve_ops._SUB_OPCODE_FOR_NAME:
        row = max(dve_ops._SUB_OPCODE_FOR_NAME.values()) + 1
        assert row < 0x20
        dve_ops._SUB_OPCODE_FOR_NAME[name] = row
    shas = {}
    for ver in ("v3", "v4"):
        s = DveOpSpec(
            name=name,
            opcode=dve_ops.get_dve_sub_opcode(name),
            uops=dve_ops.lower(spec, ver=ver),
            rd1_en=dve_ops.has_src1(spec),
        )
        shas[ver] = s.sha(ver)
    op = dve_ops.DveOp(name, spec, subdim=False, uops_sha=shas)
    if not any(o.name == name for o in dve_ops.OPS):
        dve_ops.OPS.append(op)
        dve_ops.CUSTOM_DVE_SPECS[name] = spec
    _CA_OPS = (op,)
    return _CA_OPS


def _build(repeat=1, loop_k=1, t_rows=2048, he_eng="vector", hm_eng="ts",
           xfold="v2", q_x="sync", q_e="scalar", q_w="gpsimd", q_st="gpsimd",
           unroll=12, bufs_in=4, bufs_work=4, bufs_out=4, pe_fuse=False,
           psum_grain=None, ablate=()):
    """Build the per-core Bacc module.

    he_eng: engine for he2 = x + e: 'vector' | 'pool'
    hm_eng: 'stt'  -> u = (he2 + ntgt) * w in one DVE scalar_tensor_tensor
            'act'  -> hm = Identity(he2 + ntgt) on ScalarE, u = hm*w on DVE
    xfold:  'v2'   -> v2 = t2 + x on DVE; psum = nAT@rm1 + nI@v2  (8 matmuls)
            'pe'   -> psum = nAT@rm1 + nI@t2 + nI@x               (12 matmuls)
    q_x/q_e/q_w/q_st: DMA queue for the x / e_x / W_a loads and the store:
            'sync' | 'scalar' | 'gpsimd' (SWDGE spreads over all 16 engines)
    unroll: passes per For_i iteration (amortizes the loop-boundary
            drain + semaphore reset, ~8us); loop_k must divide by it
    ablate: stages to skip for timing experiments only (output wrong):
            'dve', 'pe', 'act', 'load', 'store'
    """
    ablate = set(ablate)
    T = t_rows
    NTILES = ROWS_PER_CORE // T
    NCH = T // 512                     # matmul chunks per tile
    if psum_grain is None:
        psum_grain = 1024 if xfold == "actinit" else T
    PH = psum_grain
    psum_bufs = max(1, (16384 // (4 * PH)))   # use all 8 PSUM banks
    nc = bacc.Bacc("TRN2", target_bir_lowering=False, debug=False)

    xew = nc.declare_dram_parameter("xew", [3 * DIM, ROWS_PER_CORE], BF16, isOutput=False)
    nAT = nc.declare_dram_parameter("nAT", [DIM, DIM], BF16, isOutput=False)
    ntgt = nc.declare_dram_parameter("ntgt", [DIM, 1], F32, isOutput=False)
    out = nc.declare_dram_parameter("out", [DIM, ROWS_PER_CORE], BF16, isOutput=True)

    # [t][f, c, w]: feature f on partitions, plane c (x/e/w), row window w
    xew_t = xew.ap().rearrange("(c f) (t w) -> t f c w", c=3, w=T)
    out_t = out.ap().rearrange("f (t w) -> t f w", w=T)

    (op_rm1,) = _register_ca_ops()

    Q = {"sync": nc.sync, "scalar": nc.scalar, "gpsimd": nc.gpsimd}

    with tile.TileContext(nc) as tc:
        with (
            tc.tile_pool(name="consts", bufs=1) as consts,
            tc.tile_pool(name="inp", bufs=bufs_in) as inp,
            tc.tile_pool(name="work", bufs=bufs_work) as work,
            tc.tile_pool(name="outp", bufs=bufs_out) as outp,
            tc.tile_pool(name="psum_mm", bufs=psum_bufs, space="PSUM") as psum_mm,
        ):
            # ---- one-time constants -------------------------------------
            nat_sb = consts.tile([DIM, DIM], BF16)
            nc.sync.dma_start(out=nat_sb[:], in_=nAT.ap())

            tgt_sb = consts.tile([DIM, 1], F32)
            nc.sync.dma_start(out=tgt_sb[:], in_=ntgt.ap())

            ident = consts.tile([DIM, DIM], F32)
            masks.make_identity(nc, ident[:])
            ni_sb = consts.tile([DIM, DIM], BF16)
            nc.scalar.mul(ni_sb[:], ident[:], -1.0)

            # ---- main loop ----------------------------------------------
            def emit_pass():
                for i in range(NTILES):
                    in_tile = inp.tile([DIM, 3, T], BF16, tag="in")
                    he2 = work.tile([DIM, T], BF16, tag="he")
                    if "load" not in ablate:
                        Q[q_x].dma_start(out=in_tile[:, 0, :],
                                         in_=xew_t[i][:, 0, :])
                        if he_eng == "dma":
                            # he2 = x + e computed by the SDMA inline ALU:
                            # load x into he2, then accumulate-load e onto it.
                            Q[q_e].dma_start(out=he2[:], in_=xew_t[i][:, 0, :])
                            Q[q_e].dma_start(out=he2[:], in_=xew_t[i][:, 1, :],
                                             accum_op=mybir.AluOpType.add)
                        else:
                            Q[q_e].dma_start(out=in_tile[:, 1, :],
                                             in_=xew_t[i][:, 1, :])
                        Q[q_w].dma_start(out=in_tile[:, 2, :],
                                         in_=xew_t[i][:, 2, :])
                    x = in_tile[:, 0, :]
                    e = in_tile[:, 1, :]
                    w = in_tile[:, 2, :]

                    skip_dve = "dve" in ablate

                    u = work.tile([DIM, T], BF16, tag="u")
                    rm1 = work.tile([DIM, T], BF16, tag="rm1")
                    t2 = work.tile([DIM, T], BF16, tag="t2")
                    if PH == T:
                        mm = psum_mm.tile([DIM, T], F32, tag="mm", name="mm")
                    else:
                        mm = None
                    if not skip_dve:
                        # rm1 first: it only needs x, and it unblocks the
                        # nAT matmuls early to keep PE fed.
                        nc.vector._custom_dve(
                            op_rm1, out=rm1[:], in0=x,
                            s0=float(np.float32(-0.23549792)),
                            s1=float(np.float32(2.0017324)),
                        )
                        if hm_eng == "sttx":
                            pass  # he2 not needed: fused into the stt below
                        elif he_eng == "pool":
                            nc.gpsimd.tensor_add(he2[:], x, e)
                        elif he_eng == "vector":
                            nc.vector.tensor_add(he2[:], x, e)
                        if hm_eng == "stt":
                            nc.vector.scalar_tensor_tensor(
                                u[:], he2[:], tgt_sb[:, 0:1], w,
                                op0=mybir.AluOpType.add,
                                op1=mybir.AluOpType.mult,
                            )
                        elif hm_eng == "sttx":
                            # hm = (x + ntgt) + e in ONE DVE op (no he2 op,
                            # no ScalarE hop)
                            hm = work.tile([DIM, T], BF16, tag="hm")
                            nc.vector.scalar_tensor_tensor(
                                hm[:], x, tgt_sb[:, 0:1], e,
                                op0=mybir.AluOpType.add,
                                op1=mybir.AluOpType.add,
                            )
                            nc.vector.tensor_mul(u[:], hm[:], w)
                        elif hm_eng == "ts":
                            # hm = he2 + ntgt via DVE tensor_scalar
                            # (per-partition scalar; 2x_2p/4x eligible)
                            hm = work.tile([DIM, T], BF16, tag="hm")
                            nc.vector.tensor_scalar_add(hm[:], he2[:],
                                                        tgt_sb[:, 0:1])
                            nc.vector.tensor_mul(u[:], hm[:], w)
                        else:
                            hm = work.tile([DIM, T], BF16, tag="hm")
                            nc.scalar.add(hm[:], he2[:], tgt_sb[:, 0:1])
                            nc.vector.tensor_mul(u[:], hm[:], w)
                        nc.vector.tensor_mul(t2[:], u[:], rm1[:])
                    else:
                        nc.vector.tensor_copy(rm1[:], x)
                        nc.vector.tensor_copy(t2[:], x)

                    if xfold == "v2" and not skip_dve:
                        v2 = work.tile([DIM, T], BF16, tag="v2")
                        nc.vector.tensor_add(v2[:], t2[:], x)

                    out_tile = outp.tile([DIM, T], BF16, tag="out")
                    for pi in range(T // PH):
                        ps = slice(PH * pi, PH * (pi + 1))
                        mmp = mm if PH == T else psum_mm.tile([DIM, PH], F32,
                                                              tag="mm")
                        if "pe" in ablate:
                            nc.vector.tensor_copy(mmp[:], t2[:, ps])
                        else:
                            ai = xfold == "actinit" and not skip_dve
                            if ai:
                                # psum := -x by ScalarE; matmuls accumulate
                                # on top (no start=True reset)
                                nc.scalar.mul(mmp[:], x[:, ps], -1.0)
                            for ci in range(PH // 512):
                                cl = slice(512 * ci, 512 * (ci + 1))
                                cg = slice(PH * pi + 512 * ci,
                                           PH * pi + 512 * (ci + 1))
                                nc.tensor.matmul(mmp[:, cl], nat_sb[:],
                                                 rm1[:, cg],
                                                 start=not ai, stop=False,
                                                 skip_group_check=True)
                                if xfold == "v2" and not skip_dve:
                                    nc.tensor.matmul(mmp[:, cl], ni_sb[:],
                                                     v2[:, cg],
                                                     start=False, stop=True,
                                                     skip_group_check=True)
                                else:
                                    nc.tensor.matmul(mmp[:, cl], ni_sb[:],
                                                     t2[:, cg],
                                                     start=False, stop=ai,
                                                     skip_group_check=True)
                                    if not ai:
                                        nc.tensor.matmul(mmp[:, cl], ni_sb[:],
                                                         x[:, cg],
                                                         start=False, stop=True,
                                                         skip_group_check=True)
                        if "act" not in ablate:
                            nc.scalar.copy(out_tile[:, ps], mmp[:])
                        else:
                            nc.vector.tensor_copy(out_tile[:, ps], rm1[:, ps])

                    if "store" not in ablate:
                        Q[q_st].dma_start(out=out_t[i], in_=out_tile[:])

            if loop_k > 1:
                stag = bool(int(os.environ.get("CA_STAG", "0")))
                u_ = next(u for u in range(min(unroll, loop_k), 0, -1)
                          if loop_k % u == 0)
                if loop_k // u_ > 1:
                    with tc.For_i(0, loop_k // u_, 1, staggered_reset=stag):
                        for _ in range(u_):
                            emit_pass()
                else:
                    for _ in range(loop_k):
                        emit_pass()
            else:
                for _ in range(repeat):
                    emit_pass()

    nc.compile()
    return nc


def _make_runner(nc):
    """Cached jitted shard_map executor for a prebuilt Bacc module."""
    import jax
    from jax.experimental.shard_map import shard_map
    from jax.sharding import Mesh, PartitionSpec
    from concourse import bass2jax

    bass2jax.install_neuronx_cc_hook()

    partition_name = nc.partition_id_tensor.name if nc.partition_id_tensor else None
    in_names, out_names, out_avals, zero_shapes = [], [], [], []
    for alloc in nc.m.functions[0].allocations:
        if not isinstance(alloc, mybir.MemoryLocationSet):
            continue
        name = alloc.memorylocations[0].name
        if alloc.kind == "ExternalInput":
            if name != partition_name:
                in_names.append(name)
        elif alloc.kind == "ExternalOutput":
            out_names.append(name)
            shape = tuple(alloc.tensor_shape)
            dtype = mybir.dt.np(alloc.dtype)
            out_avals.append(jax.core.ShapedArray(shape, dtype))
            zero_shapes.append((shape, dtype))
    n_params = len(in_names)
    n_outs = len(out_names)
    bind_in_names = list(in_names) + list(out_names)
    if partition_name is not None:
        bind_in_names.append(partition_name)

    def _body(*args):
        operands = list(args)
        if partition_name is not None:
            operands.append(bass2jax.partition_id_tensor())
        outs = bass2jax._bass_exec_p.bind(
            *operands,
            out_avals=tuple(out_avals),
            in_names=tuple(bind_in_names),
            out_names=tuple(out_names),
            lowering_input_output_aliases=(),
            sim_require_finite=True,
            sim_require_nnan=True,
            nc=nc,
        )
        return tuple(outs)

    devices = jax.devices()[:NCORES]
    assert len(devices) == NCORES
    mesh = Mesh(np.asarray(devices), ("core",))
    in_specs = (PartitionSpec("core"),) * (n_params + n_outs)
    out_specs = (PartitionSpec("core"),) * n_outs
    sharded = jax.jit(
        shard_map(_body, mesh=mesh, in_specs=in_specs, out_specs=out_specs,
                  check_rep=False),
        keep_unused=True,
    )

    return {
        "fn": sharded,
        "mesh": mesh,
        "in_names": in_names,
        "out_names": out_names,
        "zero_shapes": zero_shapes,
        "n_params": n_params,
    }


def _get_runner(repeat=1, **buildkw):
    key = (repeat, tuple(sorted(buildkw.items())))
    if key not in _RUNNERS:
        _RUNNERS[key] = _make_runner(_build(repeat, **buildkw))
    return _RUNNERS[key]


def _concat_inputs(state, A, target):
    """Host-side pack: shard + transpose to feature-major bf16.

    xew[c] = [x_c^T; e_c^T; w_c^T] stacked as [3*100, 16384] per core.
    nAT = -A.T, ntgt = -target: pure constant preprocessing (O(d^2)).
    """
    s = np.asarray(state, dtype=np.float32).reshape(NCORES, ROWS_PER_CORE, 4 * DIM)
    xew = np.empty((NCORES, 3, DIM, ROWS_PER_CORE), dtype=NP_BF16)
    xew[:, 0] = s[:, :, 0:DIM].transpose(0, 2, 1)
    xew[:, 1] = s[:, :, DIM:2 * DIM].transpose(0, 2, 1)
    xew[:, 2] = s[:, :, 3 * DIM:4 * DIM].transpose(0, 2, 1)

    nat = np.ascontiguousarray((-np.asarray(A, dtype=np.float32).T)).astype(NP_BF16)
    ntg = (-np.asarray(target, dtype=np.float32))[:, None]
    return {
        "xew": xew.reshape(NCORES * 3 * DIM, ROWS_PER_CORE),
        "nAT": np.concatenate([nat] * NCORES, axis=0),
        "ntgt": np.ascontiguousarray(np.concatenate([ntg] * NCORES, axis=0)),
    }


def run_on_device(state, A, target, repeat=1, n_timed=0, **buildkw):
    """Execute; optionally time n_timed extra calls (device-resident inputs).

    Returns (dxT_global [8*100, 16384] bf16, times_s list).
    """
    import jax
    from jax.sharding import NamedSharding, PartitionSpec
    import time

    runner = _get_runner(repeat, **buildkw)
    fn = runner["fn"]
    mesh = runner["mesh"]
    shard = NamedSharding(mesh, PartitionSpec("core"))

    cat = _concat_inputs(state, A, target)
    dev_in = [jax.device_put(cat[name], shard) for name in runner["in_names"]]
    dev_z = [
        jax.device_put(np.zeros((NCORES * sh[0], *sh[1:]), dt), shard)
        for (sh, dt) in runner["zero_shapes"]
    ]
    jax.block_until_ready(dev_z)

    outs = fn(*dev_in, *dev_z)
    jax.block_until_ready(outs)
    times = []
    for _ in range(n_timed):
        t0 = time.perf_counter()
        o = fn(*dev_in, *dev_z)
        jax.block_until_ready(o)
        times.append(time.perf_counter() - t0)
    result = np.asarray(outs[0])
    return result, times


def kernel(state, A, target):
    state = np.ascontiguousarray(np.asarray(state, dtype=np.float32))
    A = np.ascontiguousarray(np.asarray(A, dtype=np.float32))
    target = np.ascontiguousarray(np.asarray(target, dtype=np.float32))
    assert state.shape == (BATCH, 4 * DIM)

    dxt, _ = run_on_device(state, A, target, repeat=1)
    # dxt: [8*100, 16384] bf16 = per-core dx^T
    dx = (
        dxt.reshape(NCORES, DIM, ROWS_PER_CORE)
        .transpose(0, 2, 1)
        .reshape(BATCH, DIM)
        .astype(np.float32)
    )
    full = np.zeros((BATCH, 4 * DIM), dtype=np.float32)
    full[:, 0:DIM] = dx
    full[:, DIM:2 * DIM] = -dx
    return full


# revision 29
# speedup vs baseline: 1.2092x; 1.2092x over previous
"""Trainium2 Bass kernel for nn_CA_event (CA_event.forward batched ODE RHS).

reference:
    x   = state[:, 0:100]
    e_x = state[:, 100:200]
    W_a = state[:, 300:400]          (W_c = state[:, 200:300] unused)
    u   = W_a * (x + e_x - target)
    s   = x^2 / (1 + x^2)
    dx  = -x + s @ A.T + u * s
    out = concat([dx, -dx, 0, 0], axis=-1)      # [B, 400]

Strategy: pure data parallel over 8 NeuronCores (batch 131072 -> 16384
rows/core), FEATURE-MAJOR bf16 layout.  Host packs the three used state
slices transposed into one bf16 tensor xew[c] = [3, 100, 16384]
(planes x / e_x / W_a; feature dim on partitions), sends the constants
nAT = -A.T (bf16) and ntgt = -target ([100,1] f32).  The rel-err gate is
2e-2; bf16 I/O keeps the L2 error ~3e-3 while cutting HBM traffic from
2000 B/row (f32, both dx and -dx stored) to 800 B/row: reads 600 B
(x,e,W_a bf16) + writes 200 B (dx bf16 only; -dx is mirrored host-side,
the W_c/W_a derivative halves are structurally zero for any input).

Per 2048-row tile [100 partitions x 2048] (default knobs):
    V:   rm1 = 1/(1+x^2) - 1 = -s       (custom DVE op, 1x: NOT-seed + 1 NR)
         he2 = x + e                    (bf16 tensor_tensor, 2x mode)
         u   = hm * w                   (2x)
         t2  = u * rm1 = -u*s           (2x)
         v2  = t2 + x                   (2x)
    ACT: hm  = he2 + (-target)          (Identity act, per-partition bias)
         out = Copy(psum) -> bf16
    PE:  psum = nAT@rm1 + nI@v2 = A@s + u*s - x = dx^T
         (512-col chunks; no transposes anywhere -- the feature-major
          layout makes the moving operands k-major natively)
    DMA: x on the sync HWDGE queue (lands first; unblocks rm1 + nAT
         matmuls), e on the scalar HWDGE queue, W_a + stores on GpSimd
         SWDGE (the only queue that spreads over all 16 SDMA engines).
    For_i timing loop unrolls 3 passes per iteration to amortize the
    loop-boundary drain + semaphore reset (~8us).

Measured on the 8-core axon TRN2: ~52-57us/pass (vs 107-132us baseline);
DVE is the saturated engine (~97%), at its 5-op floor for this math.
"""

import os
import sys

try:
    import concourse  # noqa: F401  (resolves via the environment's default path)
except ImportError:  # fall back for bare environments
    sys.path.insert(0, "/opt/trn_rl_repo")

import numpy as np
import ml_dtypes

import concourse.bass as bass
import concourse.bacc as bacc
import concourse.mybir as mybir
from concourse import tile
from concourse import masks

DIM = 100
BATCH = 131072
NCORES = 8
ROWS_PER_CORE = BATCH // NCORES          # 16384

F32 = mybir.dt.float32
BF16 = mybir.dt.bfloat16
NP_BF16 = ml_dtypes.bfloat16

_RUNNERS = {}  # key -> runner dict
_CA_OPS = None


def _register_ca_ops():
    """Register the fused custom-DVE op rm1 = 1/(1+x^2) - 1 (= -s) from x.

    Chebyshev bitwise-NOT reciprocal seed + one Newton pass on d = 1+x^2,
    minus 1; ~1e-3 rel accuracy on r, which is plenty under the 2e-2 gate.
    Same body as the baseline's proven CA_RM1_NR1.
    """
    global _CA_OPS
    if _CA_OPS is not None:
        return _CA_OPS
    from concourse import dve_ops
    from concourse.dve_spec import Spec, Src0, C0, C1, One, Bin, AluOp, sq
    from concourse.dve_uop import DveOpSpec

    dC = sq(Src0) + One
    ndC = Bin(AluOp.BITWISE_NOT, dC, dC)
    y0C = ndC * C0
    bodyC = y0C * (C1 - dC * y0C) - One

    def refC(in0, in1, s0, s1, imm2):
        d = (1.0 + in0.astype(np.float32) * in0).astype(np.float32)
        nd = (~d.view(np.int32)).view(np.float32)
        yy0 = (nd * np.float32(s0)).astype(np.float32)
        return (yy0 * (np.float32(s1) - d * yy0) - 1.0).astype(np.float32)

    name, spec = "CA_RM1_NR1", Spec(body=bodyC, reference=refC)
    if name not in dve_ops._SUB_OPCODE_FOR_NAME:
        row = max(dve_ops._SUB_OPCODE_FOR_NAME.values()) + 1
        assert row < 0x20
        dve_ops._SUB_OPCODE_FOR_NAME[name] = row
    shas = {}
    for ver in ("v3", "v4"):
        s = DveOpSpec(
            name=name,
            opcode=dve_ops.get_dve_sub_opcode(name),
            uops=dve_ops.lower(spec, ver=ver),
            rd1_en=dve_ops.has_src1(spec),
        )
        shas[ver] = s.sha(ver)
    op = dve_ops.DveOp(name, spec, subdim=False, uops_sha=shas)
    if not any(o.name == name for o in dve_ops.OPS):
        dve_ops.OPS.append(op)
        dve_ops.CUSTOM_DVE_SPECS[name] = spec
    _CA_OPS = (op,)
    return _CA_OPS


def _build(repeat=1, loop_k=1, t_rows=2048, he_eng="vector", hm_eng="act",
           xfold="v2", q_x="sync", q_e="scalar", q_w="gpsimd", q_st="gpsimd",
           unroll=12, bufs_in=4, bufs_work=4, bufs_out=4, pe_fuse=False,
           psum_grain=None, ablate=()):
    """Build the per-core Bacc module.

    he_eng: engine for he2 = x + e: 'vector' | 'pool'
    hm_eng: 'stt'  -> u = (he2 + ntgt) * w in one DVE scalar_tensor_tensor
            'act'  -> hm = Identity(he2 + ntgt) on ScalarE, u = hm*w on DVE
    xfold:  'v2'   -> v2 = t2 + x on DVE; psum = nAT@rm1 + nI@v2  (8 matmuls)
            'pe'   -> psum = nAT@rm1 + nI@t2 + nI@x               (12 matmuls)
    q_x/q_e/q_w/q_st: DMA queue for the x / e_x / W_a loads and the store:
            'sync' | 'scalar' | 'gpsimd' (SWDGE spreads over all 16 engines)
    unroll: passes per For_i iteration (amortizes the loop-boundary
            drain + semaphore reset, ~8us); loop_k must divide by it
    ablate: stages to skip for timing experiments only (output wrong):
            'dve', 'pe', 'act', 'load', 'store'
    """
    ablate = set(ablate)
    T = t_rows
    NTILES = ROWS_PER_CORE // T
    NCH = T // 512                     # matmul chunks per tile
    if psum_grain is None:
        psum_grain = 1024 if xfold == "actinit" else T
    PH = psum_grain
    psum_bufs = max(1, (16384 // (4 * PH)))   # use all 8 PSUM banks
    nc = bacc.Bacc("TRN2", target_bir_lowering=False, debug=False)

    xew = nc.declare_dram_parameter("xew", [3 * DIM, ROWS_PER_CORE], BF16, isOutput=False)
    nAT = nc.declare_dram_parameter("nAT", [DIM, DIM], BF16, isOutput=False)
    ntgt = nc.declare_dram_parameter("ntgt", [DIM, 1], F32, isOutput=False)
    out = nc.declare_dram_parameter("out", [DIM, ROWS_PER_CORE], BF16, isOutput=True)

    # [t][f, c, w]: feature f on partitions, plane c (x/e/w), row window w
    xew_t = xew.ap().rearrange("(c f) (t w) -> t f c w", c=3, w=T)
    out_t = out.ap().rearrange("f (t w) -> t f w", w=T)

    (op_rm1,) = _register_ca_ops()

    Q = {"sync": nc.sync, "scalar": nc.scalar, "gpsimd": nc.gpsimd}

    with tile.TileContext(nc) as tc:
        with (
            tc.tile_pool(name="consts", bufs=1) as consts,
            tc.tile_pool(name="inp", bufs=bufs_in) as inp,
            tc.tile_pool(name="work", bufs=bufs_work) as work,
            tc.tile_pool(name="outp", bufs=bufs_out) as outp,
            tc.tile_pool(name="psum_mm", bufs=psum_bufs, space="PSUM") as psum_mm,
        ):
            # ---- one-time constants -------------------------------------
            nat_sb = consts.tile([DIM, DIM], BF16)
            nc.sync.dma_start(out=nat_sb[:], in_=nAT.ap())

            tgt_sb = consts.tile([DIM, 1], F32)
            nc.sync.dma_start(out=tgt_sb[:], in_=ntgt.ap())

            ident = consts.tile([DIM, DIM], F32)
            masks.make_identity(nc, ident[:])
            ni_sb = consts.tile([DIM, DIM], BF16)
            nc.scalar.mul(ni_sb[:], ident[:], -1.0)

            # ---- main loop ----------------------------------------------
            def emit_pass():
                for i in range(NTILES):
                    in_tile = inp.tile([DIM, 3, T], BF16, tag="in")
                    he2 = work.tile([DIM, T], BF16, tag="he")
                    if "load" not in ablate:
                        Q[q_x].dma_start(out=in_tile[:, 0, :],
                                         in_=xew_t[i][:, 0, :])
                        if he_eng == "dma":
                            # he2 = x + e computed by the SDMA inline ALU:
                            # load x into he2, then accumulate-load e onto it.
                            Q[q_e].dma_start(out=he2[:], in_=xew_t[i][:, 0, :])
                            Q[q_e].dma_start(out=he2[:], in_=xew_t[i][:, 1, :],
                                             accum_op=mybir.AluOpType.add)
                        else:
                            Q[q_e].dma_start(out=in_tile[:, 1, :],
                                             in_=xew_t[i][:, 1, :])
                        Q[q_w].dma_start(out=in_tile[:, 2, :],
                                         in_=xew_t[i][:, 2, :])
                    x = in_tile[:, 0, :]
                    e = in_tile[:, 1, :]
                    w = in_tile[:, 2, :]

                    skip_dve = "dve" in ablate

                    u = work.tile([DIM, T], BF16, tag="u")
                    rm1 = work.tile([DIM, T], BF16, tag="rm1")
                    t2 = work.tile([DIM, T], BF16, tag="t2")
                    if PH == T:
                        mm = psum_mm.tile([DIM, T], F32, tag="mm", name="mm")
                    else:
                        mm = None
                    if not skip_dve:
                        # rm1 first: it only needs x, and it unblocks the
                        # nAT matmuls early to keep PE fed.
                        nc.vector._custom_dve(
                            op_rm1, out=rm1[:], in0=x,
                            s0=float(np.float32(-0.23549792)),
                            s1=float(np.float32(2.0017324)),
                        )
                        if hm_eng == "sttx":
                            pass  # he2 not needed: fused into the stt below
                        elif he_eng == "pool":
                            nc.gpsimd.tensor_add(he2[:], x, e)
                        elif he_eng == "vector":
                            nc.vector.tensor_add(he2[:], x, e)
                        if hm_eng == "stt":
                            nc.vector.scalar_tensor_tensor(
                                u[:], he2[:], tgt_sb[:, 0:1], w,
                                op0=mybir.AluOpType.add,
                                op1=mybir.AluOpType.mult,
                            )
                        elif hm_eng == "sttx":
                            # hm = (x + ntgt) + e in ONE DVE op (no he2 op,
                            # no ScalarE hop)
                            hm = work.tile([DIM, T], BF16, tag="hm")
                            nc.vector.scalar_tensor_tensor(
                                hm[:], x, tgt_sb[:, 0:1], e,
                                op0=mybir.AluOpType.add,
                                op1=mybir.AluOpType.add,
                            )
                            nc.vector.tensor_mul(u[:], hm[:], w)
                        elif hm_eng == "ts":
                            # hm = he2 + ntgt via DVE tensor_scalar
                            # (per-partition scalar; 2x_2p/4x eligible)
                            hm = work.tile([DIM, T], BF16, tag="hm")
                            nc.vector.tensor_scalar_add(hm[:], he2[:],
                                                        tgt_sb[:, 0:1])
                            nc.vector.tensor_mul(u[:], hm[:], w)
                        else:
                            hm = work.tile([DIM, T], BF16, tag="hm")
                            nc.scalar.add(hm[:], he2[:], tgt_sb[:, 0:1])
                            nc.vector.tensor_mul(u[:], hm[:], w)
                        nc.vector.tensor_mul(t2[:], u[:], rm1[:])
                    else:
                        nc.vector.tensor_copy(rm1[:], x)
                        nc.vector.tensor_copy(t2[:], x)

                    if xfold == "v2" and not skip_dve:
                        v2 = work.tile([DIM, T], BF16, tag="v2")
                        nc.vector.tensor_add(v2[:], t2[:], x)

                    out_tile = outp.tile([DIM, T], BF16, tag="out")
                    for pi in range(T // PH):
                        ps = slice(PH * pi, PH * (pi + 1))
                        mmp = mm if PH == T else psum_mm.tile([DIM, PH], F32,
                                                              tag="mm")
                        if "pe" in ablate:
                            nc.vector.tensor_copy(mmp[:], t2[:, ps])
                        else:
                            ai = xfold == "actinit" and not skip_dve
                            if ai:
                                # psum := -x by ScalarE; matmuls accumulate
                                # on top (no start=True reset)
                                nc.scalar.mul(mmp[:], x[:, ps], -1.0)
                            for ci in range(PH // 512):
                                cl = slice(512 * ci, 512 * (ci + 1))
                                cg = slice(PH * pi + 512 * ci,
                                           PH * pi + 512 * (ci + 1))
                                nc.tensor.matmul(mmp[:, cl], nat_sb[:],
                                                 rm1[:, cg],
                                                 start=not ai, stop=False,
                                                 skip_group_check=True)
                                if xfold == "v2" and not skip_dve:
                                    nc.tensor.matmul(mmp[:, cl], ni_sb[:],
                                                     v2[:, cg],
                                                     start=False, stop=True,
                                                     skip_group_check=True)
                                else:
                                    nc.tensor.matmul(mmp[:, cl], ni_sb[:],
                                                     t2[:, cg],
                                                     start=False, stop=ai,
                                                     skip_group_check=True)
                                    if not ai:
                                        nc.tensor.matmul(mmp[:, cl], ni_sb[:],
                                                         x[:, cg],
                                                         start=False, stop=True,
                                                         skip_group_check=True)
                        if "act" not in ablate:
                            nc.scalar.copy(out_tile[:, ps], mmp[:])
                        else:
                            nc.vector.tensor_copy(out_tile[:, ps], rm1[:, ps])

                    if "store" not in ablate:
                        Q[q_st].dma_start(out=out_t[i], in_=out_tile[:])

            if loop_k > 1:
                stag = bool(int(os.environ.get("CA_STAG", "0")))
                u_ = next(u for u in range(min(unroll, loop_k), 0, -1)
                          if loop_k % u == 0)
                if loop_k // u_ > 1:
                    with tc.For_i(0, loop_k // u_, 1, staggered_reset=stag):
                        for _ in range(u_):
                            emit_pass()
                else:
                    for _ in range(loop_k):
                        emit_pass()
            else:
                for _ in range(repeat):
                    emit_pass()

    nc.compile()
    return nc


def _make_runner(nc):
    """Cached jitted shard_map executor for a prebuilt Bacc module."""
    import jax
    from jax.experimental.shard_map import shard_map
    from jax.sharding import Mesh, PartitionSpec
    from concourse import bass2jax

    bass2jax.install_neuronx_cc_hook()

    partition_name = nc.partition_id_tensor.name if nc.partition_id_tensor else None
    in_names, out_names, out_avals, zero_shapes = [], [], [], []
    for alloc in nc.m.functions[0].allocations:
        if not isinstance(alloc, mybir.MemoryLocationSet):
            continue
        name = alloc.memorylocations[0].name
        if alloc.kind == "ExternalInput":
            if name != partition_name:
                in_names.append(name)
        elif alloc.kind == "ExternalOutput":
            out_names.append(name)
            shape = tuple(alloc.tensor_shape)
            dtype = mybir.dt.np(alloc.dtype)
            out_avals.append(jax.core.ShapedArray(shape, dtype))
            zero_shapes.append((shape, dtype))
    n_params = len(in_names)
    n_outs = len(out_names)
    bind_in_names = list(in_names) + list(out_names)
    if partition_name is not None:
        bind_in_names.append(partition_name)

    def _body(*args):
        operands = list(args)
        if partition_name is not None:
            operands.append(bass2jax.partition_id_tensor())
        outs = bass2jax._bass_exec_p.bind(
            *operands,
            out_avals=tuple(out_avals),
            in_names=tuple(bind_in_names),
            out_names=tuple(out_names),
            lowering_input_output_aliases=(),
            sim_require_finite=True,
            sim_require_nnan=True,
            nc=nc,
        )
        return tuple(outs)

    devices = jax.devices()[:NCORES]
    assert len(devices) == NCORES
    mesh = Mesh(np.asarray(devices), ("core",))
    in_specs = (PartitionSpec("core"),) * (n_params + n_outs)
    out_specs = (PartitionSpec("core"),) * n_outs
    sharded = jax.jit(
        shard_map(_body, mesh=mesh, in_specs=in_specs, out_specs=out_specs,
                  check_rep=False),
        keep_unused=True,
    )

    return {
        "fn": sharded,
        "mesh": mesh,
        "in_names": in_names,
        "out_names": out_names,
        "zero_shapes": zero_shapes,
        "n_params": n_params,
    }


def _get_runner(repeat=1, **buildkw):
    key = (repeat, tuple(sorted(buildkw.items())))
    if key not in _RUNNERS:
        _RUNNERS[key] = _make_runner(_build(repeat, **buildkw))
    return _RUNNERS[key]


def _concat_inputs(state, A, target):
    """Host-side pack: shard + transpose to feature-major bf16.

    xew[c] = [x_c^T; e_c^T; w_c^T] stacked as [3*100, 16384] per core.
    nAT = -A.T, ntgt = -target: pure constant preprocessing (O(d^2)).
    """
    s = np.asarray(state, dtype=np.float32).reshape(NCORES, ROWS_PER_CORE, 4 * DIM)
    xew = np.empty((NCORES, 3, DIM, ROWS_PER_CORE), dtype=NP_BF16)
    xew[:, 0] = s[:, :, 0:DIM].transpose(0, 2, 1)
    xew[:, 1] = s[:, :, DIM:2 * DIM].transpose(0, 2, 1)
    xew[:, 2] = s[:, :, 3 * DIM:4 * DIM].transpose(0, 2, 1)

    nat = np.ascontiguousarray((-np.asarray(A, dtype=np.float32).T)).astype(NP_BF16)
    ntg = (-np.asarray(target, dtype=np.float32))[:, None]
    return {
        "xew": xew.reshape(NCORES * 3 * DIM, ROWS_PER_CORE),
        "nAT": np.concatenate([nat] * NCORES, axis=0),
        "ntgt": np.ascontiguousarray(np.concatenate([ntg] * NCORES, axis=0)),
    }


def run_on_device(state, A, target, repeat=1, n_timed=0, **buildkw):
    """Execute; optionally time n_timed extra calls (device-resident inputs).

    Returns (dxT_global [8*100, 16384] bf16, times_s list).
    """
    import jax
    from jax.sharding import NamedSharding, PartitionSpec
    import time

    runner = _get_runner(repeat, **buildkw)
    fn = runner["fn"]
    mesh = runner["mesh"]
    shard = NamedSharding(mesh, PartitionSpec("core"))

    cat = _concat_inputs(state, A, target)
    dev_in = [jax.device_put(cat[name], shard) for name in runner["in_names"]]
    dev_z = [
        jax.device_put(np.zeros((NCORES * sh[0], *sh[1:]), dt), shard)
        for (sh, dt) in runner["zero_shapes"]
    ]
    jax.block_until_ready(dev_z)

    outs = fn(*dev_in, *dev_z)
    jax.block_until_ready(outs)
    times = []
    for _ in range(n_timed):
        t0 = time.perf_counter()
        o = fn(*dev_in, *dev_z)
        jax.block_until_ready(o)
        times.append(time.perf_counter() - t0)
    result = np.asarray(outs[0])
    return result, times


def kernel(state, A, target):
    state = np.ascontiguousarray(np.asarray(state, dtype=np.float32))
    A = np.ascontiguousarray(np.asarray(A, dtype=np.float32))
    target = np.ascontiguousarray(np.asarray(target, dtype=np.float32))
    assert state.shape == (BATCH, 4 * DIM)

    dxt, _ = run_on_device(state, A, target, repeat=1)
    # dxt: [8*100, 16384] bf16 = per-core dx^T
    dx = (
        dxt.reshape(NCORES, DIM, ROWS_PER_CORE)
        .transpose(0, 2, 1)
        .reshape(BATCH, DIM)
        .astype(np.float32)
    )
    full = np.zeros((BATCH, 4 * DIM), dtype=np.float32)
    full[:, 0:DIM] = dx
    full[:, DIM:2 * DIM] = -dx
    return full
